# revision 7
# baseline (speedup 1.0000x reference)
"""BC6H surrogate block-level decode kernel for 8 Trainium2 NeuronCores, v13.

Full-input contract: kernel(**inputs) takes the complete arrays from
setup_inputs() and returns the full (3, 4096, 4096) image.  The block
dimension (nb = 1048576) is sharded 8 ways; each core runs an identical
Bass/Tile program on its 131072-block shard.

Math (per block b, pixel p, channel c), with EU = 31248/1024:
  sig_i = sigmoid(endpoints_i)                       (4 endpoints x 3 ch)
  w     = (63*sig(idx) + clip(7*sig(idx)-3,0,1))/64  (exact LUT lerp)
  m~    = EU * softmax(logits) @ bank                (scaled soft mask)
  u'    = s2u + d32*(EU*w) + (d02 + dd*w)*m~         (u - 1.5009765625)
  hh14  = round(u') - 14          (fp32 magic-add; round==floor here)
  out   = 2^hh14 * ((u'+1.5009765625) - round(u'))

v2 design vs v1 (1.36 ms):
  * logits are pre-transposed on the HOST into [32*q+j, block] chunk
    tiles, so the PE transposes (8/supertile) and their PSUM round trip
    are gone entirely; exp() runs SBUF->SBUF on ACT.
  * bank matmul emits 17 cols/band (16 pixels + 1/EU ones-col) instead
    of 49 (num was replicated x3 channels); m~ = num * rcp(den/EU) is
    materialized 16-wide and broadcast into the 48-wide passes by AP.
  * endpoints are host-packed endpoint-major (i,c,g), so every
    coefficient op is a dense [128,96] op (v1 paid 18 ns/elem on
    inner-dim-3 strided subs).
  * NO gpsimd tensor_scalar (21.9 us/op on HW, and it stalls concurrent
    DVE ops on the shared SBUF port for ~16 us).  GpSimd gets only
    TENSOR_TENSOR adds.  The magic-round runs as a 2x-mode DVE
    tensor_scalar; frac*2^hh14 is one fused 2-src custom DVE op.
"""

import sys

sys.path.insert(0, "/opt/trn_rl_repo")

from contextlib import ExitStack

import numpy as np

import concourse.bass as bass
import concourse.tile as tile
from concourse import bacc, mybir
from concourse import bass_utils
from concourse import dve_ops
from concourse.dve_ops import DveOp
from concourse.dve_spec import (
    Spec,
    Src0,
    Src1,
    C0,
    C1,
    C2,
    One,
    relu,
    minn,
    lower,
    _has_src1,
)
from concourse.dve_uop import DveOpSpec

F32 = mybir.dt.float32
AOp = mybir.AluOpType

# ---------------------------------------------------------------- constants
NB = 1048576
N_CORES = 8
NB_CORE = NB // N_CORES            # 131072 blocks per core
G = 32                             # blocks per partition-row per supertile
ST_BLOCKS = 128 * G                # 4096 blocks per supertile
N_ST = NB_CORE // ST_BLOCKS        # 32 supertiles per core
H = W = 4096
BY = BX = 1024

EU_SCALE = 31248.0 / 1024.0        # 30.515625
EU_BIAS = 248.0 / 1024.0           # 0.2421875
FLOOR_OFF_H = 1.5009765625         # 1 + 1/1024 + 0.5 (round -> floor shift)
S2U_BIAS = EU_BIAS - FLOOR_OFF_H
MAGIC = 12582912.0                 # 1.5 * 2^23
LN2 = 0.6931471805599453

# ------------------------------------------------------- custom DVE ops
_REGISTERED = {}


def _register(name, spec):
    if name in _REGISTERED:
        return _REGISTERED[name]
    if name not in dve_ops._SUB_OPCODE_FOR_NAME:
        row = max(dve_ops._SUB_OPCODE_FOR_NAME.values()) + 1
        assert row < 0x20, "custom-DVE opcode rows exhausted"
        dve_ops._SUB_OPCODE_FOR_NAME[name] = row
    row = dve_ops._SUB_OPCODE_FOR_NAME[name]
    shas = {}
    for ver in ("v3", "v4"):
        try:
            uops = lower(spec, ver=ver)
            shas[ver] = DveOpSpec(
                name=name, opcode=row, uops=uops, rd1_en=_has_src1(spec)
            ).sha(ver)
        except Exception:
            if ver == "v3":
                raise
    op = DveOp(name, spec, subdim=False, uops_sha=shas)
    dve_ops.OPS.append(op)
    dve_ops.CUSTOM_DVE_SPECS[name] = op.spec
    _REGISTERED[name] = op
    return op


# w = s - (s - min(relu(s*c0 + c1), 1))*c2 ; c0=7, c1=-3, c2=1/64
#   = (63 s + clip(7s-3, 0, 1)) / 64
BC6W = _register(
    "BC6W_ANT",
    Spec(
        body=Src0 - (Src0 - minn(relu(Src0 * C0 + C1), One)) * C2,
        reference=lambda in0, in1, c0, c1, c2: (
            in0.astype(np.float32)
            - (
                in0.astype(np.float32)
                - np.minimum(
                    np.maximum(in0.astype(np.float32) * c0 + c1, 0.0), 1.0
                )
            )
            * c2
        ).astype(np.float32),
    ),
)


# frac = (u' + c0) - ((u' + c1) - c2) ; 1-src custom (2-src customs run
# ~2.8 cyc/elem on HW -- the fused frac*e2 variant measured 4.5 us/tile).
BC6FRAC = _register(
    "BC6FRAC_ANT",
    Spec(
        body=(Src0 + C0) - ((Src0 + C1) - C2),
        reference=lambda in0, in1, c0, c1, c2: (
            (in0.astype(np.float32) + np.float32(c0)).astype(np.float32)
            - (
                (in0.astype(np.float32) + np.float32(c1)).astype(np.float32)
                - np.float32(c2)
            ).astype(np.float32)
        ).astype(np.float32),
    ),
)


# ------------------------------------------------------- bass kernel build
def _emit_b1(nc, st, g, cv_gc_of):
    """Stage B1 (one iteration late): POOL adds, q*m~, and the CCE
    accumulate that forms u' in st["t1"]."""
    t1, t14, q, q4 = st["t1"], st["t14"], st["q"], st["q4"]

    def pv(p_t):
        return _ap4(p_t[:, :], [[16, g], [0, 3], [1, 16]])

    nc.gpsimd.tensor_add(q4, q4, cv_gc_of(st["d02"]))      # P4  POOL
    nc.gpsimd.tensor_add(t14, t14, cv_gc_of(st["s2u"]))    # P2  POOL
    nc.vector.tensor_mul(q4, q4, pv(st["mt"]))             # P5  DVE
    # P6: u' = t1 += q via the SDMA CCE adder (SBUF->SBUF accum DMA).
    # Runs on the AXI ports -- no shared-port contention with DVE/POOL.
    nc.gpsimd.dma_start(t1[:], q[:], accum_op=AOp.add)


def _emit_b2(nc, big_pool, out_pool, out, st, g):
    """Stage B2 (two iterations late): decode u' and store.  The extra
    stage gives the accumulate DMA a full supertile to land, so the
    magic-round TS never stalls on it."""
    t1 = st["t1"]
    x1 = big_pool.tile([128, 48 * g], F32, tag="hh")
    nc.scalar.activation(
        x1[:], t1[:], mybir.ActivationFunctionType.Identity,
        bias=st["magic_t"][:, 0:1], scale=1.0,
    )
    nc.scalar.activation(
        x1[:], x1[:], mybir.ActivationFunctionType.Identity,
        bias=st["nmagic14_t"][:, 0:1], scale=1.0,
    )
    e2 = big_pool.tile([128, 48 * g], F32, tag="e2")
    nc.scalar.activation(
        e2[:], x1[:], mybir.ActivationFunctionType.Exp, bias=0.0, scale=LN2
    )
    fr = big_pool.tile([128, 48 * g], F32, tag="fr")
    nc.vector._custom_dve(
        BC6FRAC, out=fr[:], in0=t1[:], s0=FLOOR_OFF_H, s1=MAGIC, imm2=MAGIC
    )
    o_t = out_pool.tile([128, 48 * g], F32, tag="o")
    nc.vector.tensor_mul(o_t[:], fr[:], e2[:])

    b0 = st["b0"]
    nc.sync.dma_start(
        out[b0 : b0 + 128 * g, :].rearrange("(r g) d -> r (g d)", g=g),
        o_t[:],
    )


def _ap4(base, dims):
    """Manual free-dim AP: keep base's partition dim, set free dims."""
    return bass.AP(base.tensor, base.offset, [list(base.ap[0])] + dims)


def build_kernel(n_st=N_ST, g=G):
    st_blocks = 128 * g
    assert g == 32

    nc = bacc.Bacc(
        "TRN2",
        target_bir_lowering=False,
        debug=False,
        enable_asserts=False,
        num_devices=1,
    )

    eps = nc.dram_tensor("eps", [n_st * 128, 12 * g], F32, kind="ExternalInput").ap()
    ixs = nc.dram_tensor("indices", [n_st * st_blocks, 16], F32, kind="ExternalInput").ap()
    lgt = nc.dram_tensor("lgt", [n_st * 128, 32 * g], F32, kind="ExternalInput").ap()
    # bank68: rows 32q+j, cols 17q..17q+15 = bank[j,:], col 17q+16 = 1/EU.
    bank = nc.dram_tensor("bank68", [128, 68], F32, kind="ExternalInput").ap()
    out = nc.dram_tensor("out", [n_st * st_blocks, 48], F32, kind="ExternalOutput").ap()

    with tile.TileContext(nc) as tc, ExitStack() as ctx:
        const_pool = ctx.enter_context(tc.tile_pool(name="const", bufs=1))
        in_pool = ctx.enter_context(tc.tile_pool(name="inp", bufs=3))
        act_pool = ctx.enter_context(tc.tile_pool(name="actp", bufs=2))
        coef_pool = ctx.enter_context(tc.tile_pool(name="coef", bufs=3))
        big_pool = ctx.enter_context(tc.tile_pool(name="big", bufs=4))
        out_pool = ctx.enter_context(tc.tile_pool(name="outp", bufs=2))
        ps_pool = ctx.enter_context(tc.tile_pool(name="ps", bufs=4, space="PSUM"))

        bank_t = const_pool.tile([128, 68], F32)
        nc.sync.dma_start(bank_t[:], bank)
        half_t = const_pool.tile([128, 1], F32)
        nc.gpsimd.memset(half_t[:], 0.5)
        s2ub_t = const_pool.tile([128, 1], F32)
        nc.gpsimd.memset(s2ub_t[:], EU_SCALE / 2.0 + S2U_BIAS)
        magic_t = const_pool.tile([128, 1], F32)
        nc.gpsimd.memset(magic_t[:], MAGIC)
        nmagic14_t = const_pool.tile([128, 1], F32)
        nc.gpsimd.memset(nmagic14_t[:], -(MAGIC + 14.0))
        prev = None
        prev2 = None

        for t in range(n_st):
            r0 = t * 128
            b0 = t * st_blocks
            # ---- loads (contiguous per partition) ----
            eps_t = in_pool.tile([128, 12 * g], F32, tag="eps")
            nc.sync.dma_start(eps_t[:], eps[r0 : r0 + 128, :])
            ixs_t = in_pool.tile([128, 16 * g], F32, tag="ixs")
            nc.sync.dma_start(
                ixs_t[:],
                ixs[b0 : b0 + st_blocks, :].rearrange("(r g) d -> r (g d)", g=g),
            )
            lgt_t = in_pool.tile([128, 32 * g], F32, tag="lgt")
            nc.sync.dma_start(lgt_t[:], lgt[r0 : r0 + 128, :])

            # ---- ACT: tanh + exp live in ONE table set (exp_and_others),
            # so there are no ACT_TABLE_LOADs after the first supertile.
            # sigmoid(x) = 0.5*tanh(x/2) + 0.5; the 0.5's fold into wt, s2u
            # and the bank ones-column (m~ carries EU/2).
            sige = act_pool.tile([128, 12 * g], F32, tag="sige")
            nc.scalar.activation(
                sige[:], eps_t[:], mybir.ActivationFunctionType.Tanh, scale=0.5
            )
            sigx = act_pool.tile([128, 16 * g], F32, tag="sigx")
            nc.scalar.activation(
                sigx[:], ixs_t[:], mybir.ActivationFunctionType.Tanh, scale=0.5
            )
            lgs = act_pool.tile([128, 32 * g], F32, tag="lgs")
            nc.scalar.activation(
                lgs[:], lgt_t[:], mybir.ActivationFunctionType.Exp
            )

            # ---- PE: 8 chunk matmuls into 2 PSUM tiles (4 chunks each) ----
            ps_h = []
            for h in range(2):
                ps = ps_pool.tile([128, 272], F32, tag=f"ps{h}")
                for cc in range(4):
                    ch = 4 * h + cc
                    nc.tensor.matmul(
                        ps[:, 68 * cc : 68 * cc + 68],
                        lgs[:, 128 * ch : 128 * (ch + 1)],
                        bank_t[:],
                        start=True,
                        stop=True,
                    )
                ps_h.append(ps)

            # ---- m~ = num * (EU/den): rcp of den'(=den/EU), then 16-wide ----
            rcpe = coef_pool.tile([128, 32], F32, tag="rcpe")
            for h in range(2):
                nc.vector.reciprocal(
                    rcpe[:, 16 * h : 16 * h + 16],
                    _ap4(ps_h[h][:, 16:], [[17, 16]]),
                )
            mt = coef_pool.tile([128, 16 * g], F32, tag="mt")
            for h in range(2):
                nc.vector.tensor_mul(
                    _ap4(mt[:, 256 * h :], [[16, 16], [1, 16]]),
                    _ap4(ps_h[h][:, :], [[17, 16], [1, 16]]),
                    _ap4(rcpe[:, 16 * h :], [[1, 16], [0, 16]]),
                )

            # ---- s = 0.5*T+0.5, w (custom DVE), w~ = (EU/2)*w on ACT ----
            sx = coef_pool.tile([128, 16 * g], F32, tag="sx")
            nc.scalar.activation(
                sx[:], sigx[:], mybir.ActivationFunctionType.Identity,
                bias=half_t[:, 0:1], scale=0.5,
            )
            w_t = coef_pool.tile([128, 16 * g], F32, tag="w")
            nc.vector._custom_dve(
                BC6W, out=w_t[:], in0=sx[:], s0=7.0, s1=-3.0, imm2=1.0 / 64.0
            )
            wt_t = coef_pool.tile([128, 16 * g], F32, tag="wt")
            nc.scalar.mul(wt_t[:], w_t[:], EU_SCALE / 2.0)

            # ---- coefficients on tanh values T = 2*sig-1 ----
            # eps is (i,c,g)-major.  d32/dd (DVE-mul sources) stay (c,g);
            # s2u/d02 (POOL-add sources) are written (g,c)-transposed so
            # their broadcast views coalesce to 2-dim APs for the Q7s.
            cg = 3 * g

            def esl(i):
                return sige[:, cg * i : cg * (i + 1)]

            def esl_cg(i):
                return _ap4(sige[:, cg * i :], [[32, 3], [1, 32]])

            d32 = coef_pool.tile([128, 2 * cg], F32, tag="d32")
            d13 = d32[:, cg : 2 * cg]
            nc.vector.tensor_sub(
                d32[:, :],
                _ap4(sige[:, 3 * cg :], [[-2 * cg, 2], [1, cg]]),
                _ap4(sige[:, 2 * cg :], [[cg, 2], [1, cg]]),
            )
            d02 = coef_pool.tile([128, cg], F32, tag="d02")
            d02_t = _ap4(d02[:, :], [[1, 3], [3, 32]])
            nc.vector.tensor_sub(d02_t, esl_cg(0), esl_cg(2))
            dd = coef_pool.tile([128, cg], F32, tag="dd")
            nc.vector.tensor_sub(
                _ap4(dd[:, :], [[32, 3], [1, 32]]),
                _ap4(d13[:, :], [[32, 3], [1, 32]]),
                _ap4(d02[:, :], [[1, 3], [3, 32]]),
            )
            s2u = coef_pool.tile([128, cg], F32, tag="s2u")
            nc.scalar.activation(
                _ap4(s2u[:, :], [[1, 3], [3, 32]]), esl_cg(2),
                mybir.ActivationFunctionType.Identity,
                bias=s2ub_t[:, 0:1], scale=EU_SCALE / 2.0,
            )

            # ---- big passes: u' = (s2u + d32*w~) + (d02 + dd*w)*m~ ----
            # coef (c,g) -> (g,c,p); pixel (g,p) -> (g,c,p) broadcast views
            def cv(c_t):     # (c,g)-stored coef -> (g,c,p) view (DVE muls)
                return _ap4(c_t[:, :], [[1, g], [g, 3], [0, 16]])

            def cv_gc(c_t):  # (g,c)-stored coef -> coalescable (POOL adds)
                return _ap4(c_t[:, :], [[3, g], [1, 3], [0, 16]])

            def pv(p_t):
                return _ap4(p_t[:, :], [[16, g], [0, 3], [1, 16]])

            t1 = big_pool.tile([128, 48 * g], F32, tag="t1")
            t14 = _ap4(t1[:, :], [[48, g], [16, 3], [1, 16]])
            q = big_pool.tile([128, 48 * g], F32, tag="q")
            q4 = _ap4(q[:, :], [[48, g], [16, 3], [1, 16]])

            nc.vector.tensor_mul(t14, cv(d32), pv(wt_t))       # P1  DVE
            nc.vector.tensor_mul(q4, cv(dd), pv(w_t))          # P3  DVE

            cur = dict(t1=t1, t14=t14, q=q, q4=q4, s2u=s2u, d02=d02,
                       mt=mt, b0=b0, magic_t=magic_t, nmagic14_t=nmagic14_t)
            if prev is not None:
                _emit_b1(nc, prev, g, cv_gc_of=cv_gc)
            if prev2 is not None:
                _emit_b2(nc, big_pool, out_pool, out, prev2, g)
            prev2 = prev
            prev = cur
        _emit_b1(nc, prev, g, cv_gc_of=cv_gc)
        _emit_b2(nc, big_pool, out_pool, out, prev2, g)
        _emit_b2(nc, big_pool, out_pool, out, prev, g)

    nc.compile()
    return nc


# ------------------------------------------------------- host-side driver
_NC_CACHE = {}


def _get_nc():
    if "nc" not in _NC_CACHE:
        _NC_CACHE["nc"] = build_kernel()
    return _NC_CACHE["nc"]


def make_in_maps(endpoints, indices, partition_logits, partition_bank, nb=NB):
    """Shard + pack host inputs into the 8 per-core input dicts."""
    bank = np.ascontiguousarray(partition_bank.astype(np.float32))
    b68 = np.zeros((128, 68), dtype=np.float32)
    for qq in range(4):
        b68[32 * qq : 32 * qq + 32, 17 * qq : 17 * qq + 16] = bank
        b68[32 * qq : 32 * qq + 32, 17 * qq + 16] = 2.0 / EU_SCALE

    ep_all = endpoints.astype(np.float32).reshape(nb, 4, 3)
    ix_all = indices.astype(np.float32)
    lg_all = partition_logits.astype(np.float32)
    nbc = nb // N_CORES
    in_maps = []
    for c in range(N_CORES):
        sl = slice(c * nbc, (c + 1) * nbc)
        # endpoints: (t, r, g, i, ch) -> (t, r, i, ch, g): eslice(i) dense
        ep_c = np.ascontiguousarray(
            ep_all[sl]
            .reshape(N_ST, 128, G, 4, 3)
            .transpose(0, 1, 3, 4, 2)
            .reshape(N_ST * 128, 12 * G)
        )
        # logits: block-in-st = 32n + 4ch + q -> rows (q,j), cols (ch,n)
        lg_c = np.ascontiguousarray(
            lg_all[sl]
            .reshape(N_ST, 128, 8, 4, 32)
            .transpose(0, 3, 4, 2, 1)
            .reshape(N_ST * 128, 32 * G)
        )
        in_maps.append(
            {
                "eps": ep_c,
                "indices": np.ascontiguousarray(ix_all[sl]),
                "lgt": lg_c,
                "bank68": b68,
            }
        )
    return in_maps


def blocks_to_img(blocks):
    """[NB, 48] c-major blocks -> (3, H, W) image."""
    return (
        blocks.reshape(BY, BX, 3, 4, 4)
        .transpose(2, 0, 3, 1, 4)
        .reshape(3, H, W)
        .astype(np.float32)
    )


def kernel(endpoints, indices, partition_logits, partition_bank, weight_lut):
    endpoints = np.asarray(endpoints, dtype=np.float32)
    indices = np.asarray(indices, dtype=np.float32)
    partition_logits = np.asarray(partition_logits, dtype=np.float32)
    partition_bank = np.asarray(partition_bank, dtype=np.float32)
    assert endpoints.shape[0] == NB

    in_maps = make_in_maps(endpoints, indices, partition_logits, partition_bank)
    nc = _get_nc()
    res = bass_utils.run_bass_kernel_spmd(
        nc, in_maps, core_ids=list(range(N_CORES))
    )
    blocks = np.concatenate(
        [res.results[c]["out"] for c in range(N_CORES)], axis=0
    )
    return blocks_to_img(blocks)
